# revision 1
# baseline (speedup 1.0000x reference)
"""Trainium2 Bass kernel for nn_AvgPoolVectorsPerWSI (segment-mean over groups).

Math: x [N=2048, M=512, 7, 7], idx [N] in [0,64)
  out[g, m] = mean over {n: idx[n]==g} and spatial of x[n, m, :, :]  -> [64, 512, 1, 1]

Strategy (no collectives needed):
  - Shard over M: core k handles an m-slice of 64 channels. Each core reads
    its x slice [2048, 64, 49] (25.7 MB) once -> memory-bound (~63-68 us/core
    DMA stream; the NC-pair shares one HBM stack, so ~358 GB/s/core sustained).
  - All compute is fp32-exact. The per-tile work is split across engines so
    both stay well under the ~3.95 us/tile DMA pace:
      * TensorE, m-channels [0, MC): fused segment-sum directly on raw x,
          psum_big[g, (m,j)] += w[n, g]^T @ x[n, (m,j)]
      * VectorE, m-channels [MC, 64): spatial j-reduce to xs[n, m], then a
        tiny fp32 matmul accumulates psum_small[g, m] += w[n, g]^T @ xs[n, m]
    with w the scale-weighted one-hot (scale = 1/(count_g*49)), generated
    ON DEVICE from a 74 KB aux tensor (iota/scale/idx) so the HBM stream is
    just x.
  - Epilogue (pipelined with the last tile's matmul chunks): j-reduce
    psum_big -> out[:, :MC] in three m-chunks, copy psum_small -> out[:, MC:],
    DMA out [64, 64]. Host concatenates the 8 results along m.

Raw Block implementation (not Tile): the walrus matmul/DMA lowerings only
accept ONE attached sync-wait per instruction; standalone wait_ge
instructions sidestep that.

DMA-completion semaphores: tile t uses sem t % BUFS with a cumulative
threshold. A shared counter is only safe because a tile's sem is reused
(t+BUFS) strictly after tile t was consumed (the slot-reuse wait orders the
re-issue); with fewer sems than BUFS, a straggling SDMA engine (engine 15
runs ~20% slow in some periods) could satisfy tile t's count with later
tiles' stripes while t is still in flight.
"""

from contextlib import ExitStack

import numpy as np

import concourse.bass as bass
import concourse.mybir as mybir
from concourse.bass_utils import run_bass_kernel_spmd

N = 2048          # samples
M = 512           # channels
HW = 49           # spatial (7*7)
G = 64            # groups
CORES = 8
ML = M // CORES   # 64 channels per core
F = ML * HW       # 3136 floats per (n, core)
P = 128           # partitions per tile
NT = N // P       # 16 n-tiles
BUFS = 8          # x-tile buffer depth == number of DMA semaphores

MC = 30           # m-channels handled by TensorE (raw fused matmul)
MV = ML - MC      # m-channels handled by VectorE reduce
FC = MC * HW      # 1470 raw columns through the PE
# fp32 matmul chunks must stay within one 2KB PSUM bank -> 512-col chunks
CHUNKS = [(c * 512, min((c + 1) * 512, FC)) for c in range((FC + 511) // 512)]
NCH = len(CHUNKS)
# epilogue sub-reduce m-chunks of psum_big, aligned to the matmul chunks:
# sub-chunk i needs matmul chunks 0..need_i of the last tile (pe_big counts
# one inc per chunk per tile, in chunk order).
SUBRED = []
for _mlo, _mhi in ((0, 10), (10, 20), (20, MC)):
    _need = next(i for i, (lo, hi) in enumerate(CHUNKS) if hi >= _mhi * HW)
    SUBRED.append((_mlo, _mhi, (NT - 1) * NCH + _need + 1))

F32 = mybir.dt.float32


def _build():
    nc = bass.Bass(trn_type="TRN2", target_bir_lowering=False)
    x_ext = nc.declare_dram_parameter("x", [N, F], F32, isOutput=False)
    # aux[:, 0:64] iota row, aux[:, 64:128] scale row, aux[:, 128:144] idx
    aux_ext = nc.declare_dram_parameter("aux", [P, G + G + NT], F32,
                                        isOutput=False)
    out_ext = nc.declare_dram_parameter("out", [G, ML], F32, isOutput=True)

    x_t = x_ext.ap().rearrange("(t p) f -> t p f", p=P)  # [16, 128, 3136]

    with ExitStack() as ctx:
        x_buf = ctx.enter_context(nc.sbuf_tensor([P, BUFS * F], F32))
        xs_buf = ctx.enter_context(nc.sbuf_tensor([P, BUFS * MV], F32))
        aux_sb = ctx.enter_context(nc.sbuf_tensor([P, G + G + NT], F32))
        w_sb = ctx.enter_context(nc.sbuf_tensor([P, NT * G], F32))
        out_sb = ctx.enter_context(nc.sbuf_tensor([G, ML], F32))
        psum_big = ctx.enter_context(nc.psum_tensor([G, FC], F32))
        psum_small = ctx.enter_context(nc.psum_tensor([G, MV], F32))
        dma_x = [
            ctx.enter_context(nc.semaphore(name=f"dma_x{s}"))
            for s in range(BUFS)
        ]
        dma_a = ctx.enter_context(nc.semaphore())   # +16 when aux resident
        dma_o = ctx.enter_context(nc.semaphore())   # +16 when out written
        wg_sem = ctx.enter_context(nc.semaphore())  # +1 when w generated
        red_sem = ctx.enter_context(nc.semaphore())  # +1 per tile j-reduce
        pe_big = ctx.enter_context(nc.semaphore())   # +1 per big matmul chunk
        pe_tile = ctx.enter_context(nc.semaphore())  # +1 per tile (small mm)
        fin_sem = ctx.enter_context(nc.semaphore())  # +4 when out_sb ready
        block = ctx.enter_context(nc.Block())

        def xwait(engine, t):
            engine.wait_ge(dma_x[t % BUFS], 16 * (t // BUFS + 1))

        # ---- DMA program (SP / HWDGE, FIFO) ----
        @block.sync
        def _(sync):
            def xdma(t):
                if t >= BUFS:
                    # slot reuse: the small matmul is ordered after both the
                    # j-reduce and the big matmuls of its tile
                    sync.wait_ge(pe_tile, t - BUFS + 1)
                slot = t % BUFS
                sync.dma_start(
                    out=x_buf[:, slot * F:(slot + 1) * F], in_=x_t[t]
                ).then_inc(dma_x[slot], 16)

            xdma(0)
            sync.dma_start(out=aux_sb[:, :], in_=aux_ext.ap()).then_inc(dma_a, 16)
            for t in range(1, NT):
                xdma(t)
            sync.wait_ge(fin_sem, 4)
            sync.dma_start(out=out_ext.ap(), in_=out_sb[:, :]).then_inc(dma_o, 16)
            sync.wait_ge(dma_o, 16)

        # ---- VectorE: w generation, j-reduction, epilogue ----
        @block.vector
        def _(vector):
            # generate the scale-weighted one-hot from idx:
            #   w[p, t*G+g] = (idx[t*128+p] == g) * scale[g]
            vector.wait_ge(dma_a, 16)
            for t in range(NT):
                wg = vector.scalar_tensor_tensor(
                    out=w_sb[:, t * G:(t + 1) * G],
                    in0=aux_sb[:, 0:G],
                    scalar=aux_sb[:, 2 * G + t:2 * G + t + 1],
                    in1=aux_sb[:, G:2 * G],
                    op0=mybir.AluOpType.is_equal,
                    op1=mybir.AluOpType.mult,
                )
            wg.then_inc(wg_sem, 1)

            for t in range(NT):
                xwait(vector, t)
                if t >= BUFS:
                    # xs slot reuse: wait until tile t-BUFS consumed by PE
                    vector.wait_ge(pe_tile, t - BUFS + 1)
                slot = t % BUFS
                vector.tensor_reduce(
                    out=xs_buf[:, slot * MV:(slot + 1) * MV],
                    in_=x_buf[:, slot * F + FC:(slot + 1) * F].rearrange(
                        "p (m j) -> p m j", j=HW
                    ),
                    axis=mybir.AxisListType.X,
                    op=mybir.AluOpType.add,
                ).then_inc(red_sem, 1)

            # epilogue: j-reduce psum_big in m-chunks as the last tile's
            # matmul chunks complete; copy psum_small
            for mlo, mhi, need in SUBRED:
                vector.wait_ge(pe_big, need)
                vector.tensor_reduce(
                    out=out_sb[:, mlo:mhi],
                    in_=psum_big[:, mlo * HW:mhi * HW].rearrange(
                        "p (m j) -> p m j", j=HW
                    ),
                    axis=mybir.AxisListType.X,
                    op=mybir.AluOpType.add,
                ).then_inc(fin_sem, 1)
            vector.wait_ge(pe_tile, NT)
            vector.tensor_copy(
                out_sb[:, MC:ML], psum_small[:, :]
            ).then_inc(fin_sem, 1)

        # ---- TensorE: segment-sum accumulation (fp32) ----
        @block.tensor
        def _(tensor):
            tensor.wait_ge(wg_sem, 1)
            for t in range(NT):
                xwait(tensor, t)
                slot = t % BUFS
                wt = w_sb[:, t * G:(t + 1) * G]
                for lo, hi in CHUNKS:
                    tensor.matmul(
                        out=psum_big[:, lo:hi],
                        lhsT=wt,
                        rhs=x_buf[:, slot * F + lo:slot * F + hi],
                        start=(t == 0),
                        stop=(t == NT - 1),
                    ).then_inc(pe_big, 1)
                tensor.wait_ge(red_sem, t + 1)
                tensor.matmul(
                    out=psum_small[:, :],
                    lhsT=wt,
                    rhs=xs_buf[:, slot * MV:(slot + 1) * MV],
                    start=(t == 0),
                    stop=(t == NT - 1),
                ).then_inc(pe_tile, 1)

    return nc


def _prepare(x, idx):
    x = np.asarray(x)
    if x.dtype != np.float32:
        x = x.astype(np.float32)
    idx = np.asarray(idx).astype(np.int64)
    counts = np.bincount(idx, minlength=G).astype(np.float64)
    scale = np.where(counts > 0, 1.0 / (counts * HW), 0.0).astype(np.float32)
    aux = np.zeros((P, G + G + NT), np.float32)
    aux[:, 0:G] = np.arange(G, dtype=np.float32)[None, :]
    aux[:, G:2 * G] = scale[None, :]
    aux[:, 2 * G:] = idx.reshape(NT, P).T.astype(np.float32)
    xr = x.reshape(N, M, HW)
    in_maps = []
    for k in range(CORES):
        shard = np.ascontiguousarray(xr[:, k * ML:(k + 1) * ML, :]).reshape(N, F)
        in_maps.append({"x": shard, "aux": aux})
    return in_maps


def run(x, tensor_list_assignmentindices, trace=False):
    in_maps = _prepare(x, tensor_list_assignmentindices)
    nc = _build()
    res = run_bass_kernel_spmd(nc, in_maps, core_ids=list(range(CORES)), trace=trace)
    outs = [np.asarray(r["out"]) for r in res.results]
    out = np.concatenate(outs, axis=1)  # [G, M]
    return out.reshape(G, M, 1, 1).astype(np.float32), res.exec_time_ns


def kernel(**inputs):
    out, _ = run(inputs["x"], inputs["tensor_list_assignmentindices"], trace=False)
    return out

